# revision 9
# baseline (speedup 1.0000x reference)
"""Trainium2 Bass kernel for nn_AttentionAggregator (segment_reduce).

Math: out[b, g] = sum_{j in group g} softmax_g(att)[j] * feat[b, flat_idx[j]]
    = (feat @ W)[b, g]   with W[k, g] = sum_{j in g, flat_idx[j] = k} attn[j]

The segment softmax and the scatter that builds W involve only the tiny
index/weight tensors, so they run on host.  The heavy part — the
(4096 x 4096) @ (4096 x 1024) product — runs on 8 NeuronCores with the
batch axis sharded 512 rows per core (embarrassingly parallel, no
collectives).  Each core computes outT = W^T-blocks x featT via 256
accumulating fp32r matmuls (lhsT = W k/g-tile, rhs = featT k-tile).

Self-contained: hardcodes shapes from the problem spec; no sibling imports.
"""

import numpy as np

B = 4096
NG = 4096
G = 1024
N_CORES = 8
B_LOC = B // N_CORES          # 512 batch rows per core
P = 128                       # SBUF/PE partitions
KT = NG // P                  # 32 contraction tiles
GT = G // P                   # 8 output-group tiles

_NC_CACHE = {}


def _host_softmax_scatter(att_weights, flat_idx, segment_ids, num_segments):
    """Per-segment softmax of att_weights, scatter-added into dense W (NG, G)."""
    aw = np.asarray(att_weights, dtype=np.float32)
    seg = np.asarray(segment_ids, dtype=np.int64)
    idx = np.asarray(flat_idx, dtype=np.int64)
    n_seg = int(num_segments)

    seg_max = np.full(n_seg, -np.inf, dtype=np.float32)
    np.maximum.at(seg_max, seg, aw)
    ex = np.exp(aw - seg_max[seg])
    denom = np.zeros(n_seg, dtype=np.float32)
    np.add.at(denom, seg, ex)
    attn = ex / denom[seg]

    w = np.zeros((NG, n_seg), dtype=np.float32)
    np.add.at(w, (idx, seg), attn)
    return w


def build_nc():
    """SPMD single-core program: outT(1024, 512) = W(4096, 1024)^T @ featT(4096, 512)."""
    import concourse.mybir as mybir
    from concourse import bacc
    from concourse.tile import TileContext

    F32 = mybir.dt.float32
    F16 = mybir.dt.float16

    nc = bacc.Bacc("TRN2", target_bir_lowering=False, debug=False)
    # Host pre-tiles both operands (fp16) into the exact SBUF layouts so
    # every DMA is fully contiguous per partition.
    # feat_t[p, k, b] = feat[core_b0 + b, k*128 + p]
    feat_t = nc.dram_tensor("featT", [P, KT, B_LOC], F16, kind="ExternalInput")
    # wmat[p, k, g] = W[k*128 + p, g]   (k-major so W streams in k-chunks)
    wmat = nc.dram_tensor("wmat", [P, KT, G], F16, kind="ExternalInput")
    out_t = nc.dram_tensor("outT", [G, B_LOC], F32, kind="ExternalOutput")

    # k-major lockstep: W and feat stream together in k-chunks; all 8 psum
    # banks accumulate their g-tile simultaneously, so the PE starts as soon
    # as the first small chunk lands and never waits for a full-tensor load.
    CHUNKS = [2, 2, 4, 8, 8, 8]         # k-tiles per chunk (sums to KT)

    with TileContext(nc) as tc:
        with (
            tc.tile_pool(name="fp", bufs=3) as fp,
            tc.tile_pool(name="wp", bufs=3) as wp,
            tc.tile_pool(name="pp", bufs=8, space="PSUM") as pp,
            tc.tile_pool(name="op", bufs=2) as op,
        ):
            # PE warm-up: the HAM clock gate keeps PE at 1.2 GHz until it has
            # been busy ~3.4us.  Run dummy matmuls while the first DMAs are in
            # flight so the real matmuls start at 2.4 GHz.
            dummy = fp.tile([P, P], F16, tag="dummy", bufs=1)
            nc.vector.memset(dummy, 0)
            ps_all = [
                pp.tile([P, B_LOC], mybir.dt.float32, name=f"ps{gt}", tag="ps")
                for gt in range(GT)
            ]
            for _ in range(36):
                nc.tensor.matmul(
                    ps_all[GT - 1][:, :P], lhsT=dummy, rhs=dummy,
                    start=True, stop=True,
                )

            k0 = 0
            for ci, ck in enumerate(CHUNKS):
                # feat chunk rides the Scalar HWDGE ring, W chunk the Sync
                # ring, so neither stream queues behind the other.
                f_sb = fp.tile([P, ck, B_LOC], F16, tag="f", padded_shape=[P, 8, B_LOC])
                nc.scalar.dma_start(f_sb, feat_t[:, k0 : k0 + ck, :])
                w_sb = wp.tile([P, ck, G], F16, tag="w", padded_shape=[P, 8, G])
                nc.sync.dma_start(w_sb, wmat[:, k0 : k0 + ck, :])
                for kl in range(ck):
                    k = k0 + kl
                    for gt in range(GT):
                        nc.tensor.matmul(
                            ps_all[gt],
                            lhsT=w_sb[:, kl, gt * P : (gt + 1) * P],
                            rhs=f_sb[:, kl, :],
                            start=(k == 0),
                            stop=(k == KT - 1),
                        )
                k0 += ck

            for gt in range(GT):
                o_sb = op.tile([P, B_LOC], F32, tag="o")
                # alternate copy engine so the final psum drains pipeline
                # across Scalar and Vector
                if gt % 2 == 0:
                    nc.scalar.copy(o_sb, ps_all[gt])
                else:
                    nc.vector.tensor_copy(o_sb, ps_all[gt])
                nc.scalar.dma_start(out_t[gt * P : (gt + 1) * P, :], o_sb)
    nc.compile()
    return nc


def make_in_maps(gene_set_features, wmat):
    feat = np.asarray(gene_set_features, dtype=np.float32).astype(np.float16)
    # (P, KT, G): wmat_tiled[p, k, g] = W[k*128 + p, g]
    w_tiled = np.ascontiguousarray(
        wmat.astype(np.float16).reshape(KT, P, G).transpose(1, 0, 2)
    )
    in_maps = []
    for c in range(N_CORES):
        shard = feat[c * B_LOC : (c + 1) * B_LOC, :]  # (B_LOC, NG)
        # (P, KT, B_LOC): feat_tiled[p, k, b] = shard[b, k*128 + p]
        feat_tiled = np.ascontiguousarray(
            shard.T.reshape(KT, P, B_LOC).transpose(1, 0, 2)
        )
        in_maps.append({"featT": feat_tiled, "wmat": w_tiled})
    return in_maps


def kernel(gene_set_features, att_weights, flat_idx, segment_ids, num_segments):
    from concourse.bass_utils import run_bass_kernel_spmd

    wmat = _host_softmax_scatter(att_weights, flat_idx, segment_ids, num_segments)
    in_maps = make_in_maps(gene_set_features, wmat)

    if "nc" not in _NC_CACHE:
        _NC_CACHE["nc"] = build_nc()
    nc = _NC_CACHE["nc"]

    res = run_bass_kernel_spmd(nc, in_maps, core_ids=list(range(N_CORES)))

    out = np.empty((B, G), dtype=np.float32)
    for c in range(N_CORES):
        out[c * B_LOC : (c + 1) * B_LOC, :] = res.results[c]["outT"].T
    return out


# revision 12
# speedup vs baseline: 1.1918x; 1.1918x over previous
"""Trainium2 Bass kernel for nn_AttentionAggregator (segment_reduce).

Math: out[b, g] = sum_{j in group g} softmax_g(att)[j] * feat[b, flat_idx[j]]
    = (feat @ W)[b, g]   with W[k, g] = sum_{j in g, flat_idx[j] = k} attn[j]

The segment softmax and the scatter that builds W involve only the tiny
index/weight tensors, so they run on host.  The heavy part — the
(4096 x 4096) @ (4096 x 1024) product — runs on 8 NeuronCores with the
batch axis sharded 512 rows per core (embarrassingly parallel, no
collectives).  Each core computes outT = W^T-blocks x featT via 256
accumulating fp32r matmuls (lhsT = W k/g-tile, rhs = featT k-tile).

Self-contained: hardcodes shapes from the problem spec; no sibling imports.
"""

import numpy as np

B = 4096
NG = 4096
G = 1024
N_CORES = 8
B_LOC = B // N_CORES          # 512 batch rows per core
P = 128                       # SBUF/PE partitions
KT = NG // P                  # 32 contraction tiles
GT = G // P                   # 8 output-group tiles

_NC_CACHE = {}


def _host_softmax_scatter(att_weights, flat_idx, segment_ids, num_segments):
    """Per-segment softmax of att_weights, scatter-added into dense W (NG, G)."""
    aw = np.asarray(att_weights, dtype=np.float32)
    seg = np.asarray(segment_ids, dtype=np.int64)
    idx = np.asarray(flat_idx, dtype=np.int64)
    n_seg = int(num_segments)

    seg_max = np.full(n_seg, -np.inf, dtype=np.float32)
    np.maximum.at(seg_max, seg, aw)
    ex = np.exp(aw - seg_max[seg])
    denom = np.zeros(n_seg, dtype=np.float32)
    np.add.at(denom, seg, ex)
    attn = ex / denom[seg]

    w = np.zeros((NG, n_seg), dtype=np.float32)
    np.add.at(w, (idx, seg), attn)
    return w


def build_nc():
    """SPMD single-core program: outT(1024, 512) = W(4096, 1024)^T @ featT(4096, 512)."""
    import concourse.mybir as mybir
    from concourse import bacc
    from concourse.tile import TileContext

    F32 = mybir.dt.float32
    F16 = mybir.dt.float16

    nc = bacc.Bacc("TRN2", target_bir_lowering=False, debug=False)
    # Host pre-tiles both operands (fp16) into the exact SBUF layouts so
    # every DMA is fully contiguous per partition.
    # feat_t[p, k, b] = feat[core_b0 + b, k*128 + p]
    feat_t = nc.dram_tensor("featT", [P, KT, B_LOC], F16, kind="ExternalInput")
    # wmat[p, k, g] = W[k*128 + p, g]   (k-major so W streams in k-chunks)
    wmat = nc.dram_tensor("wmat", [P, KT, G], F16, kind="ExternalInput")
    out_t = nc.dram_tensor("outT", [G, B_LOC], F32, kind="ExternalOutput")

    # k-major lockstep: W and feat stream together in k-chunks; all 8 psum
    # banks accumulate their g-tile simultaneously, so the PE starts as soon
    # as the first small chunk lands and never waits for a full-tensor load.
    # Uniform 2-tile chunks keep the DMA permanently ahead of the PE
    # (0.75 MB / ~2.4us per chunk vs 16 MMs / ~3.5us on the PE).
    CHUNK = 2
    K_TAIL = 8                          # final k-tiles run gt-major (below)

    with TileContext(nc) as tc:
        with (
            tc.tile_pool(name="fp", bufs=7) as fp,
            tc.tile_pool(name="wp", bufs=7) as wp,
            tc.tile_pool(name="pp", bufs=8, space="PSUM") as pp,
            tc.tile_pool(name="op", bufs=2) as op,
        ):
            # PE warm-up: the HAM clock gate keeps PE at 1.2 GHz until it has
            # been busy ~3.4us.  Run dummy matmuls while the first DMAs are in
            # flight so the real matmuls start at 2.4 GHz.
            dummy = fp.tile([P, P], F16, tag="dummy", bufs=1)
            nc.vector.memset(dummy, 0)
            ps_all = [
                pp.tile([P, B_LOC], mybir.dt.float32, name=f"ps{gt}", tag="ps")
                for gt in range(GT)
            ]
            for _ in range(36):
                nc.tensor.matmul(
                    ps_all[GT - 1][:, :P], lhsT=dummy, rhs=dummy,
                    start=True, stop=True,
                )

            # stream all chunks; keep tiles of the final K_TAIL k-tiles live
            tail_tiles = {}
            for k0 in range(0, KT, CHUNK):
                # feat chunk rides the Scalar HWDGE ring, W chunk the Sync
                # ring, so neither stream queues behind the other.
                f_sb = fp.tile([P, CHUNK, B_LOC], F16, tag="f")
                nc.scalar.dma_start(f_sb, feat_t[:, k0 : k0 + CHUNK, :])
                w_sb = wp.tile([P, CHUNK, G], F16, tag="w")
                nc.sync.dma_start(w_sb, wmat[:, k0 : k0 + CHUNK, :])
                if k0 >= KT - K_TAIL:
                    tail_tiles[k0] = (f_sb, w_sb)
                    continue
                for kl in range(CHUNK):
                    k = k0 + kl
                    for gt in range(GT):
                        nc.tensor.matmul(
                            ps_all[gt],
                            lhsT=w_sb[:, kl, gt * P : (gt + 1) * P],
                            rhs=f_sb[:, kl, :],
                            start=(k == 0),
                            stop=False,
                        )

            # final K_TAIL k-tiles run gt-major, so psums complete staggered
            # and each copy+store overlaps the remaining gt's matmuls.
            for gt in range(GT):
                for k0, (f_sb, w_sb) in tail_tiles.items():
                    for kl in range(CHUNK):
                        k = k0 + kl
                        nc.tensor.matmul(
                            ps_all[gt],
                            lhsT=w_sb[:, kl, gt * P : (gt + 1) * P],
                            rhs=f_sb[:, kl, :],
                            start=False,
                            stop=(k == KT - 1),
                        )
                o_sb = op.tile([P, B_LOC], F32, tag="o")
                # alternate copy engine so the final psum drains pipeline
                # across Scalar and Vector
                if gt % 2 == 0:
                    nc.scalar.copy(o_sb, ps_all[gt])
                else:
                    nc.vector.tensor_copy(o_sb, ps_all[gt])
                nc.sync.dma_start(out_t[gt * P : (gt + 1) * P, :], o_sb)
    nc.compile()
    return nc


def make_in_maps(gene_set_features, wmat):
    feat = np.asarray(gene_set_features, dtype=np.float32).astype(np.float16)
    # (P, KT, G): wmat_tiled[p, k, g] = W[k*128 + p, g]
    w_tiled = np.ascontiguousarray(
        wmat.astype(np.float16).reshape(KT, P, G).transpose(1, 0, 2)
    )
    in_maps = []
    for c in range(N_CORES):
        shard = feat[c * B_LOC : (c + 1) * B_LOC, :]  # (B_LOC, NG)
        # (P, KT, B_LOC): feat_tiled[p, k, b] = shard[b, k*128 + p]
        feat_tiled = np.ascontiguousarray(
            shard.T.reshape(KT, P, B_LOC).transpose(1, 0, 2)
        )
        in_maps.append({"featT": feat_tiled, "wmat": w_tiled})
    return in_maps


def kernel(gene_set_features, att_weights, flat_idx, segment_ids, num_segments):
    from concourse.bass_utils import run_bass_kernel_spmd

    wmat = _host_softmax_scatter(att_weights, flat_idx, segment_ids, num_segments)
    in_maps = make_in_maps(gene_set_features, wmat)

    if "nc" not in _NC_CACHE:
        _NC_CACHE["nc"] = build_nc()
    nc = _NC_CACHE["nc"]

    res = run_bass_kernel_spmd(nc, in_maps, core_ids=list(range(N_CORES)))

    out = np.empty((B, G), dtype=np.float32)
    for c in range(N_CORES):
        out[c * B_LOC : (c + 1) * B_LOC, :] = res.results[c]["outT"].T
    return out
